# revision 18
# baseline (speedup 1.0000x reference)
"""Trainium2 Bass kernel for ConvolutionalMultiheadAttention.

Model (reference):
  q = rotary(conv3x3(x, wq) + bq); k = rotary(conv3x3(x, wk) + bk)
  v = conv3x3(x, wv) + bv
  qk = (q @ k^T) / sqrt(C)            per (batch, head)        -> output #2
  out = conv3x3(softmax(qk) @ v, wo) + bo                      -> output #1

Shapes: x (2, 512, 32, 32), w* (512, 512, 3, 3), num_heads=8, S=1024, hd=64.

Sharding (8 cores): each core owns (batch b, head-pair hp) = one of 8
(2 batches x 4 head-pairs).  Per core:
  - q/k/v convs for its 128 channels (2 heads) over its batch: matmuls with
    M=128 (out channels), K=128 ci-chunks, N=512 spatial, 9 shifted taps.
    Conv inputs live in a zero-padded 34x33 flat spatial layout so every
    tap is a strided window read + dense contiguous PSUM write.
  - rotary via a 128x128 pair-rotation matmul + DVE elementwise.
  - scores (q-major) -> raw qk output; scores^T (k-major) -> exp -> both
    the softmax@V contraction and the denominator (ones-matmul), then
    normalize with a reciprocal+mul during PSUM->SBUF eviction.
  - partial o-conv over its 128 input channels, all 512 out channels.
Host sums the 4 per-batch o-conv partials and concatenates qk slices.
The 1/sqrt(C) score scale is folded into wq/bq on the host.

All matmuls run as float32r (full fp32 data, fast weight-load path): at
moving-dim >= 256 this streams 1 row/cycle like bf16, with fp32 precision.
"""

import sys

for _p in ("/opt/trn_rl_repo", "/root/.axon_site/_ro/trn_rl_repo"):
    if _p not in sys.path:
        sys.path.append(_p)

import math

import numpy as np

N_CORES = 8
B, C, H, W = 2, 512, 32, 32
S = H * W  # 1024
NH = 8
HD = C // NH  # 64
THETA = 10000.0
PW = 33  # padded row stride (one shared zero column between rows)
PSP = 34 * PW + 2  # padded spatial length: 34 rows x 33 cols + AP-slack

_OFFS = [(dy, dx) for dy in (-1, 0, 1) for dx in (-1, 0, 1)]


def _off_index(dy, dx):
    return (dy + 1) * 3 + (dx + 1)


_PROGRAM = None


def _build_program():
    import concourse.bass as bass  # noqa: F401
    import concourse.tile as tile
    from concourse import bacc, mybir

    f32 = mybir.dt.float32
    f32r = mybir.dt.float32r
    AF = mybir.ActivationFunctionType

    nc = bacc.Bacc("TRN2", target_bir_lowering=False, debug=False)

    # ---- DRAM I/O ----------------------------------------------------------
    x_d = nc.dram_tensor("x", [128, 4, PSP], f32, kind="ExternalInput").ap()
    wq_d = nc.dram_tensor("wq", [128, 4, 9, 128], f32, kind="ExternalInput").ap()
    wk_d = nc.dram_tensor("wk", [128, 4, 9, 128], f32, kind="ExternalInput").ap()
    wv_d = nc.dram_tensor("wv", [128, 4, 9, 128], f32, kind="ExternalInput").ap()
    wo_d = nc.dram_tensor("wo", [128, 9, 512], f32, kind="ExternalInput").ap()
    cos_d = nc.dram_tensor("cos", [128, S], f32, kind="ExternalInput").ap()
    sin_d = nc.dram_tensor("sin", [128, S], f32, kind="ExternalInput").ap()
    pt_d = nc.dram_tensor("pt", [128, 128], f32, kind="ExternalInput").ap()
    id2_d = nc.dram_tensor("id2", [128, 64], f32, kind="ExternalInput").ap()
    bq_d = nc.dram_tensor("bq", [128, 1], f32, kind="ExternalInput").ap()
    bk_d = nc.dram_tensor("bk", [128, 1], f32, kind="ExternalInput").ap()
    bv_d = nc.dram_tensor("bv", [128, 1], f32, kind="ExternalInput").ap()

    qk_d = nc.dram_tensor("qk", [2, S, S], f32, kind="ExternalOutput").ap()
    op_d = nc.dram_tensor("op", [128, 4, S], f32, kind="ExternalOutput").ap()

    def win(buf_flat, s16, dy, dx):
        """(128, 16, 32) strided window of a padded (., PSP) flat buffer."""
        base = (16 * s16 + dy + 1) * PW + dx + 1
        return buf_flat[:, base : base + 16 * PW].rearrange(
            "p (r c) -> p r c", c=PW
        )[:, :, 0:W]

    with tile.TileContext(nc) as tc:
        from contextlib import ExitStack

        with ExitStack() as ctx:
            consts = ctx.enter_context(tc.tile_pool(name="consts", bufs=1))
            tmp = ctx.enter_context(tc.tile_pool(name="tmp", bufs=2))
            scp = ctx.enter_context(tc.tile_pool(name="scp", bufs=3))
            exppool = ctx.enter_context(tc.tile_pool(name="exppool", bufs=3))
            rdp = ctx.enter_context(tc.tile_pool(name="rdp", bufs=2))
            osp = ctx.enter_context(tc.tile_pool(name="osp", bufs=3))
            psg = ctx.enter_context(tc.tile_pool(name="psg", bufs=2, space="PSUM"))
            psacc = ctx.enter_context(tc.tile_pool(name="psacc", bufs=2, space="PSUM"))

            # ---- load constants / inputs ----------------------------------
            x_sb = consts.tile([128, 4, PSP], f32)
            for cc in range(4):
                nc.sync.dma_start(
                    out=x_sb[:, cc, :].bitcast(f32r), in_=x_d[:, cc, :].bitcast(f32r)
                )
            wq_sb = consts.tile([128, 4, 9, 128], f32)
            wk_sb = consts.tile([128, 4, 9, 128], f32)
            wv_sb = consts.tile([128, 4, 9, 128], f32)
            for w_sb, w_d in ((wq_sb, wq_d), (wk_sb, wk_d), (wv_sb, wv_d)):
                for cc in range(4):
                    nc.sync.dma_start(
                        out=w_sb[:, cc].bitcast(f32r), in_=w_d[:, cc].bitcast(f32r)
                    )
            cos_sb = consts.tile([128, S], f32)
            sin_sb = consts.tile([128, S], f32)
            nc.sync.dma_start(out=cos_sb[:], in_=cos_d)
            nc.sync.dma_start(out=sin_sb[:], in_=sin_d)
            pt_sb = consts.tile([128, 128], f32)
            nc.sync.dma_start(out=pt_sb[:].bitcast(f32r), in_=pt_d.bitcast(f32r))
            id2_sb = consts.tile([128, 64], f32)
            nc.sync.dma_start(out=id2_sb[:], in_=id2_d)
            bq_sb = consts.tile([128, 1], f32)
            bk_sb = consts.tile([128, 1], f32)
            bv_sb = consts.tile([128, 1], f32)
            for t, d in ((bq_sb, bq_d), (bk_sb, bk_d), (bv_sb, bv_d)):
                nc.sync.dma_start(out=t[:], in_=d)
            wo_sb = consts.tile([128, 9, 512], f32)
            for oi in range(9):
                nc.sync.dma_start(
                    out=wo_sb[:, oi].bitcast(f32r), in_=wo_d[:, oi].bitcast(f32r)
                )
            ones_sb = consts.tile([128, 128], f32)
            nc.vector.tensor_scalar(
                ones_sb[:].bitcast(f32r),
                pt_sb[:],
                0.0,
                1.0,
                op0=mybir.AluOpType.mult,
                op1=mybir.AluOpType.add,
            )

            q_sb = consts.tile([128, S], f32)
            k_sb = consts.tile([128, S], f32)
            v_sb = consts.tile([128, S], f32)
            qr_sb = consts.tile([128, S], f32)
            kr_sb = consts.tile([128, S], f32)
            vt0_sb = consts.tile([128, 8, 128], f32)
            vt1_sb = consts.tile([128, 8, 128], f32)
            for vt_sb in (vt0_sb, vt1_sb):
                for kc in range(8):
                    nc.vector.tensor_scalar(
                        vt_sb[:, kc, :].bitcast(f32r),
                        pt_sb[:],
                        0.0,
                        None,
                        op0=mybir.AluOpType.mult,
                    )
            attn_pad = consts.tile([128, PSP], f32)
            nc.vector.tensor_scalar(
                attn_pad[:].bitcast(f32r),
                x_sb[:, 0, :],
                0.0,
                None,
                op0=mybir.AluOpType.mult,
            )

            def conv_qkv(w_sb, b_sb, out_sb):
                """3x3 conv, 512 -> 128 channels, via 72 shifted matmuls."""
                for s16 in range(2):
                    ps = psg.tile([128, 512], f32, tag="big", name="convps")
                    n = 0
                    for cc in range(4):
                        for dy, dx in _OFFS:
                            oi = _off_index(dy, dx)
                            nc.tensor.matmul(
                                ps[:],
                                lhsT=w_sb[:, cc, oi, :].bitcast(f32r),
                                rhs=win(x_sb[:, cc, :], s16, dy, dx).bitcast(f32r),
                                start=(n == 0),
                                stop=(n == 35),
                            )
                            n += 1
                    nc.vector.tensor_scalar_add(
                        out_sb[:, s16 * 512 : (s16 + 1) * 512].bitcast(f32r),
                        ps[:],
                        b_sb[:],
                    )

            conv_qkv(wq_sb, bq_sb, q_sb)
            conv_qkv(wk_sb, bk_sb, k_sb)
            conv_qkv(wv_sb, bv_sb, v_sb)

            # ---- rotary on q, k -------------------------------------------
            def rotary(src_sb, dst_sb):
                t1 = tmp.tile([128, S], f32, tag="rt", name="rott1")
                t2 = tmp.tile([128, S], f32, tag="rt", name="rott2")
                nc.vector.tensor_mul(t1[:], src_sb[:], cos_sb[:])
                for j in range(2):
                    rps = psg.tile([128, 512], f32, tag="big", name="rotps")
                    nc.tensor.matmul(
                        rps[:],
                        lhsT=pt_sb[:].bitcast(f32r),
                        rhs=src_sb[:, j * 512 : (j + 1) * 512].bitcast(f32r),
                        start=True,
                        stop=True,
                    )
                    nc.vector.tensor_mul(
                        t2[:, j * 512 : (j + 1) * 512],
                        rps[:],
                        sin_sb[:, j * 512 : (j + 1) * 512],
                    )
                nc.vector.tensor_add(dst_sb[:].bitcast(f32r), t1[:], t2[:])

            rotary(q_sb, qr_sb)
            rotary(k_sb, kr_sb)

            # ---- transpose v per head: vT[kpos, d] ------------------------
            for h, vt_sb in ((0, vt0_sb), (1, vt1_sb)):
                b0 = 64 * h
                for kc in range(8):
                    vtps = psg.tile([128, 64], f32, tag="big", name="vtps")
                    nc.tensor.matmul(
                        vtps[:],
                        lhsT=v_sb[b0 : b0 + 64, kc * 128 : (kc + 1) * 128],
                        rhs=id2_sb[b0 : b0 + 64, :],
                        is_transpose=True,
                        start=True,
                        stop=True,
                    )
                    nc.vector.tensor_copy(
                        vt_sb[:, kc, b0 : b0 + 64].bitcast(f32r), vtps[:]
                    )

            # ---- attention per head ---------------------------------------
            for h, vt_sb in ((0, vt0_sb), (1, vt1_sb)):
                b0 = 64 * h
                qh = qr_sb[b0 : b0 + 64, :]
                kh = kr_sb[b0 : b0 + 64, :]

                # raw scores (q-major) -> qk output
                for qc in range(8):
                    sps = psg.tile([128, S], f32, tag="big", name="sps")
                    for j in range(2):
                        nc.tensor.matmul(
                            sps[:, j * 512 : (j + 1) * 512],
                            lhsT=qh[:, qc * 128 : (qc + 1) * 128].bitcast(f32r),
                            rhs=kh[:, j * 512 : (j + 1) * 512].bitcast(f32r),
                            start=True,
                            stop=True,
                        )
                    ssb = scp.tile([128, S], f32, tag="sc", name="ssb")
                    nc.vector.tensor_copy(ssb[:], sps[:])
                    nc.sync.dma_start(
                        out=qk_d[h, qc * 128 : (qc + 1) * 128, :], in_=ssb[:]
                    )

                # transposed scores -> exp -> A@V and denominator
                attn_ps = psacc.tile([128, S], f32, tag="acc", name="attnps")
                den_ps = psacc.tile([128, S], f32, tag="acc", name="denps")
                for kc in range(8):
                    stps = psg.tile([128, S], f32, tag="big", name="stps")
                    for j in range(2):
                        nc.tensor.matmul(
                            stps[:, j * 512 : (j + 1) * 512],
                            lhsT=kh[:, kc * 128 : (kc + 1) * 128].bitcast(f32r),
                            rhs=qh[:, j * 512 : (j + 1) * 512].bitcast(f32r),
                            start=True,
                            stop=True,
                        )
                    ex = exppool.tile([128, S], f32, tag="ex", name="ex")
                    nc.scalar.activation(ex[:].bitcast(f32r), stps[:], func=AF.Exp)
                    for j in range(2):
                        nc.tensor.matmul(
                            attn_ps[:, j * 512 : (j + 1) * 512],
                            lhsT=vt_sb[:, kc, :].bitcast(f32r),
                            rhs=ex[:, j * 512 : (j + 1) * 512].bitcast(f32r),
                            start=(kc == 0),
                            stop=(kc == 7),
                        )
                        nc.tensor.matmul(
                            den_ps[:, j * 512 : (j + 1) * 512],
                            lhsT=ones_sb[:].bitcast(f32r),
                            rhs=ex[:, j * 512 : (j + 1) * 512].bitcast(f32r),
                            start=(kc == 0),
                            stop=(kc == 7),
                        )
                rden = rdp.tile([128, S], f32, tag="rd", name="rden")
                nc.vector.reciprocal(rden[b0 : b0 + 64, :], den_ps[b0 : b0 + 64, :])
                # write normalized attn into the zero-padded conv layout
                apad = attn_pad[:, PW + 1 : PW + 1 + 32 * PW].rearrange(
                    "p (r c) -> p r c", c=PW
                )[b0 : b0 + 64, :, 0:W]
                nc.vector.tensor_mul(
                    apad.bitcast(f32r),
                    attn_ps[b0 : b0 + 64, :].rearrange("p (r c) -> p r c", c=W),
                    rden[b0 : b0 + 64, :].rearrange("p (r c) -> p r c", c=W),
                )

            # ---- partial o-conv: 128 ci (this core) -> 512 co -------------
            for mc in range(4):
                for s16 in range(2):
                    ps = psg.tile([128, 512], f32, tag="big", name="opsum")
                    n = 0
                    for dy, dx in _OFFS:
                        oi = _off_index(dy, dx)
                        nc.tensor.matmul(
                            ps[:],
                            lhsT=wo_sb[:, oi, mc * 128 : (mc + 1) * 128].bitcast(f32r),
                            rhs=win(attn_pad[:, :], s16, dy, dx).bitcast(f32r),
                            start=(n == 0),
                            stop=(n == 8),
                        )
                        n += 1
                    osb = osp.tile([128, 512], f32, tag="os", name="osb")
                    nc.vector.tensor_copy(osb[:], ps[:])
                    nc.sync.dma_start(
                        out=op_d[:, mc, s16 * 512 : (s16 + 1) * 512], in_=osb[:]
                    )

    nc.compile()
    return nc


def _get_program():
    global _PROGRAM
    if _PROGRAM is None:
        _PROGRAM = _build_program()
    return _PROGRAM


def _host_constants():
    inv = 1.0 / (THETA ** (np.arange(0, HD, 2, dtype=np.float64) / HD))  # (32,)
    invf = np.repeat(inv, 2)  # (64,) per-dim freq
    ang = invf[:, None] * np.arange(S, dtype=np.float64)[None, :]  # (64, S)
    cos128 = np.tile(np.cos(ang), (2, 1)).astype(np.float32)
    sin128 = np.tile(np.sin(ang), (2, 1)).astype(np.float32)

    # P^T for rot = P @ x (pair rotation), blockdiag over the 2 heads
    pt = np.zeros((128, 128), np.float32)
    for base in (0, 64):
        for i in range(0, 64, 2):
            pt[base + i + 1, base + i] = -1.0  # rot[2i] = -x[2i+1]
            pt[base + i, base + i + 1] = 1.0  # rot[2i+1] = x[2i]

    id2 = np.zeros((128, 64), np.float32)
    id2[:64] = np.eye(64, dtype=np.float32)
    id2[64:] = np.eye(64, dtype=np.float32)
    return cos128, sin128, pt, id2


def _pad_spatial(img):
    """(n, 128, 32, 32) -> (n, 128, PSP) zero-padded 34x33 flat layout."""
    n = img.shape[0]
    out = np.zeros((n, 128, PSP), np.float32)
    v = out[:, :, : 34 * PW].reshape(n, 128, 34, PW)
    v[:, :, 1:33, 1:33] = img
    return out


def _prep_w_qkv(w, hp, scale=1.0):
    """(512co, 512ci, 3, 3) -> this core's (128p_ci, 4cc, 9off, 128co)."""
    ws = w[hp * 128 : (hp + 1) * 128] * scale  # (128co, 512ci, 3, 3)
    arr = ws.transpose(1, 2, 3, 0)  # (512ci, 3, 3, 128co)
    arr = arr.reshape(4, 128, 3, 3, 128).transpose(1, 0, 2, 3, 4)
    return np.ascontiguousarray(arr.reshape(128, 4, 9, 128), dtype=np.float32)


def _prep_wo(wo, hp):
    """(512co, 512ci, 3, 3) -> this core's (128p_ci, 9off, 512co)."""
    ws = wo[:, hp * 128 : (hp + 1) * 128]  # (512co, 128ci, 3, 3)
    arr = ws.transpose(1, 2, 3, 0)  # (128ci, 3, 3, 512co)
    return np.ascontiguousarray(arr.reshape(128, 9, 512), dtype=np.float32)


def kernel(x, wq, bq, wk, bk, wv, bv, wo, bo, num_heads):
    from concourse.bass_utils import run_bass_kernel_spmd

    x = np.asarray(x, np.float32)
    wq = np.asarray(wq, np.float32)
    wk = np.asarray(wk, np.float32)
    wv = np.asarray(wv, np.float32)
    wo = np.asarray(wo, np.float32)
    bq = np.asarray(bq, np.float32)
    bk = np.asarray(bk, np.float32)
    bv = np.asarray(bv, np.float32)
    bo = np.asarray(bo, np.float32)
    assert int(num_heads) == NH

    nc = _get_program()
    cos128, sin128, pt, id2 = _host_constants()
    scale = 1.0 / math.sqrt(C)

    xpad = [
        _pad_spatial(x[b].reshape(4, 128, 32, 32)).transpose(1, 0, 2) for b in range(B)
    ]
    in_maps = []
    for c in range(N_CORES):
        b, hp = divmod(c, 4)
        in_maps.append(
            {
                "x": np.ascontiguousarray(xpad[b]),
                "wq": _prep_w_qkv(wq, hp, scale),
                "wk": _prep_w_qkv(wk, hp),
                "wv": _prep_w_qkv(wv, hp),
                "wo": _prep_wo(wo, hp),
                "cos": cos128,
                "sin": sin128,
                "pt": pt,
                "id2": id2,
                "bq": np.ascontiguousarray(
                    (bq[hp * 128 : (hp + 1) * 128] * scale).reshape(128, 1), np.float32
                ),
                "bk": np.ascontiguousarray(
                    bk[hp * 128 : (hp + 1) * 128].reshape(128, 1)
                ),
                "bv": np.ascontiguousarray(
                    bv[hp * 128 : (hp + 1) * 128].reshape(128, 1)
                ),
            }
        )

    res = run_bass_kernel_spmd(nc, in_maps, list(range(N_CORES)))

    qk = np.empty((B, NH, S, S), np.float32)
    out_acc = np.zeros((B, C, S), np.float64)
    for c in range(N_CORES):
        b, hp = divmod(c, 4)
        r = res.results[c]
        qk[b, 2 * hp : 2 * hp + 2] = r["qk"]
        out_acc[b] += r["op"].transpose(1, 0, 2).reshape(C, S)
    out = (out_acc + bo.astype(np.float64)[None, :, None]).astype(np.float32)
    return out.reshape(B, C, H, W), qk
